# revision 1
# baseline (speedup 1.0000x reference)
"""GroupGMM Trainium2 kernel.

Computes, for B=8192 samples with soft group-mixture weights over G=32 groups:
    logits = einsum("bi,gio,bg->bo", x, W_pi, g) + g @ b_pi        [B, 16]
    loc    = einsum(... W_mu ...)   + g @ b_mu                     [B, 512]
    scale  = softplus(einsum(... W_sigma ...) + g @ b_sigma)+1e-7  [B, 512]
    out    = concat([logits, loc, scale], -1)                      [B, 1040]

Strategy: data-parallel over batch across 8 NeuronCores (1024 rows each).
The group einsum is folded into one matmul with contraction K = G*I = 16384
via z[b,(g,i)] = g[b,g] * x[b,i]. Per 128-sample chunk, z^T K-tiles are
built on the Vector engine (x^T tile * broadcast gate row, both bf16,
host-pre-transposed/broadcast), and the PE accumulates all 128 K-tiles
into PSUM. PSUM capacity (8 banks) fits mu+sigma accumulators for 3
sample-chunks, so the batch is processed in 3 sweeps ([0..2],[3..5],[6..7])
with the weight K-tiles re-streamed from HBM per sweep on the sync HWDGE
queue (all other traffic uses the gpsimd queue so the W stream is never
blocked). The bias term g @ b_cat is precomputed on the host and added at
drain time on DVE; sigma gets softplus via ACT Exp+Ln (one shared table).
"""

import numpy as np
import ml_dtypes

import concourse.bass as bass
import concourse.tile as tile
from concourse import bacc, mybir
from concourse.bass_utils import run_bass_kernel_spmd

B, I, G, C, D = 8192, 512, 32, 16, 32
CD = C * D                      # 512
OUT_W = C + 2 * CD              # 1040
NCORES = 8
BLOC = B // NCORES              # 1024
KTOT = G * I                    # 16384
NKT = KTOT // 128               # 128 K-tiles
NMC = BLOC // 128               # 8 sample chunks per core
SWEEPS = [[0, 1, 2], [3, 4, 5], [6, 7]]

BF16 = mybir.dt.bfloat16
F32 = mybir.dt.float32

_cache: dict = {}


def _build_program():
    if "nc" in _cache:
        return _cache["nc"]
    from contextlib import ExitStack

    nc = bacc.Bacc("TRN2", target_bir_lowering=False, debug=False)

    xt_d = nc.dram_tensor("xt", [I, BLOC], BF16, kind="ExternalInput")
    gb_d = nc.dram_tensor("gb", [G, 128, BLOC], BF16, kind="ExternalInput")
    w_d = nc.dram_tensor("w", [NKT, 128, OUT_W], BF16, kind="ExternalInput")
    bias_d = nc.dram_tensor("bias", [BLOC, OUT_W], F32, kind="ExternalInput")
    out_d = nc.dram_tensor("out", [BLOC, OUT_W], F32, kind="ExternalOutput")

    with tile.TileContext(nc) as tc, ExitStack() as ctx:
        res = ctx.enter_context(tc.tile_pool(name="res", bufs=1))
        wp = ctx.enter_context(tc.tile_pool(name="wp", bufs=6))
        zp = ctx.enter_context(tc.tile_pool(name="zp", bufs=8))
        op = ctx.enter_context(tc.tile_pool(name="op", bufs=3))
        bp = ctx.enter_context(tc.tile_pool(name="bp", bufs=4))
        pp = ctx.enter_context(tc.tile_pool(name="pp", bufs=1, space="PSUM"))

        # Startup-critical loads go on the sync HWDGE queue ahead of the W
        # stream: the first gate tile and x^T block 0; x^T blocks 1-3 are
        # interleaved with the first W tiles so the opening matmul group
        # never waits behind a megabyte of resident loads.
        gb_t = [None] * G
        gb_t[0] = res.tile([128, BLOC], BF16, name="gbt0", tag="gbt0")
        nc.sync.dma_start(gb_t[0][:], gb_d[0])
        xt_t = []
        for ib in range(I // 128):
            t = res.tile([128, BLOC], BF16, name=f"xtt{ib}", tag=f"xtt{ib}")
            xt_t.append(t)
        nc.sync.dma_start(xt_t[0][:], xt_d[0:128, :])

        carry_z: dict = {}

        def gen_z(s, kt, mcs):
            gi = kt // 4
            ib = kt % 4
            m0 = mcs[0] * 128
            mw = len(mcs) * 128
            zt = zp.tile([128, mw], BF16, name=f"zt{s}_{kt}", tag="zt")
            nc.vector.tensor_mul(zt[:], xt_t[ib][:, m0:m0 + mw],
                                 gb_t[gi][:, m0:m0 + mw])
            return zt

        for s, mcs in enumerate(SWEEPS):
            # ppi gets the 8th PSUM bank as a second slot so the next sweep
            # never waits on this sweep's pi drain.
            ppi = pp.tile([128, 16 * len(mcs)], F32, name=f"ppi{s}",
                          tag="ppi", bufs=2)
            pmu, psg, bt = {}, {}, {}
            for j, mc in enumerate(mcs):
                pmu[mc] = pp.tile([128, CD], F32, name=f"pmu{s}_{j}",
                                  tag="pmu", bufs=3)
                psg[mc] = pp.tile([128, CD], F32, name=f"psg{s}_{j}",
                                  tag="psg", bufs=3)

            for kt in range(NKT):
                gi = kt // 4
                ib = kt % 4
                if s == 0 and 1 <= kt <= 3:
                    nc.sync.dma_start(xt_t[kt][:],
                                      xt_d[kt * 128:(kt + 1) * 128, :])
                if s == 0 and ib == 0 and gi + 1 < G:
                    # Load gate tiles lazily on the fast queue, paced one
                    # group ahead of use, so PE isn't stuck behind 8.4MB of
                    # resident loads at startup.
                    t = res.tile([128, BLOC], BF16, name=f"gbt{gi + 1}",
                                 tag=f"gbt{gi + 1}")
                    nc.sync.dma_start(t[:], gb_d[gi + 1])
                    gb_t[gi + 1] = t
                if kt == 16:
                    # Bias tiles for this sweep's drain. On the sync queue
                    # mid-sweep: HWDGE executes in order, so they can't jump
                    # ahead of startup-critical loads on the shared DMA
                    # engines (gpsimd would issue them immediately).
                    for j2, mc2 in enumerate(mcs):
                        bt[mc2] = bp.tile([128, OUT_W], F32,
                                          name=f"bt{s}_{j2}", tag="bt")
                        nc.sync.dma_start(
                            bt[mc2][:],
                            bias_d[mc2 * 128:(mc2 + 1) * 128, :])
                wt = wp.tile([128, OUT_W], BF16, name=f"wt{s}_{kt}", tag="wt")
                nc.sync.dma_start(wt[:], w_d[kt])
                zt = carry_z.pop((s, kt), None)
                if zt is None:
                    zt = gen_z(s, kt, mcs)
                first = kt == 0
                last = kt == NKT - 1
                if last:
                    # Final K-tile: run the sigma matmuls first so psg is
                    # ready earliest — its drain (add→Exp→Ln) is the long
                    # serial chain at the end of the sweep.
                    for j, mc in enumerate(mcs):
                        lhs = zt[:, j * 128:(j + 1) * 128]
                        nc.tensor.matmul(psg[mc][:], lhs, wt[:, C + CD:],
                                         start=False, stop=True)
                    for j, mc in enumerate(mcs):
                        lhs = zt[:, j * 128:(j + 1) * 128]
                        nc.tensor.matmul(pmu[mc][:], lhs, wt[:, C:C + CD],
                                         start=False, stop=True)
                        nc.tensor.matmul(ppi[:, j * 16:(j + 1) * 16], lhs,
                                         wt[:, 0:C], start=False, stop=True,
                                         skip_group_check=True)
                    continue
                for j, mc in enumerate(mcs):
                    lhs = zt[:, j * 128:(j + 1) * 128]
                    nc.tensor.matmul(pmu[mc][:], lhs, wt[:, C:C + CD],
                                     start=first, stop=False)
                    nc.tensor.matmul(psg[mc][:], lhs, wt[:, C + CD:],
                                     start=first, stop=False)
                    # start=True marks the whole 2KB bank pending-zero, so
                    # only the first matmul into the shared pi bank sets it;
                    # later slices' first writes overwrite via pending-zero.
                    nc.tensor.matmul(ppi[:, j * 16:(j + 1) * 16], lhs,
                                     wt[:, 0:C], start=(first and j == 0),
                                     stop=False, skip_group_check=True)

            # Queue the next sweep's first z-tiles on DVE ahead of the drain
            # work so PE can restart immediately at the sweep boundary.
            if s + 1 < len(SWEEPS):
                for kt in range(3):
                    carry_z[(s + 1, kt)] = gen_z(s + 1, kt, SWEEPS[s + 1])

            # Drain, phase-batched so ACT runs exp,exp,..,ln,ln,.. — the
            # act-table chooser puts Exp and Ln in different function sets,
            # and interleaving them costs a 1.3us table reload per call.
            # softplus(v) = ln(exp(v) + 1); the reference's +1e-7 is dropped
            # (5e-7 relative effect, far below bf16 noise).
            ots, ets = {}, {}
            for j, mc in enumerate(mcs):
                # mu-add first frees this pmu slot for the next sweep's
                # opening matmul; ei-add right after feeds ACT and frees psg.
                ot = op.tile([128, OUT_W], F32, name=f"ot{s}_{j}", tag="ot")
                nc.vector.tensor_add(ot[:, C:C + CD], pmu[mc][:],
                                     bt[mc][:, C:C + CD])
                ei = op.tile([128, CD], F32, name=f"ei{s}_{j}", tag="ei",
                             bufs=3)
                nc.vector.tensor_add(ei[:], psg[mc][:], bt[mc][:, C + CD:])
                ots[mc] = ot
                ets[mc] = ei
            for j, mc in enumerate(mcs):
                et = op.tile([128, CD], F32, name=f"et{s}_{j}", tag="et",
                             bufs=3)
                nc.scalar.activation(et[:], ets[mc][:],
                                     mybir.ActivationFunctionType.Exp)
                ets[mc] = et
            for j, mc in enumerate(mcs):
                ot = ots[mc]
                nc.vector.tensor_add(ot[:, 0:C], ppi[:, j * 16:(j + 1) * 16],
                                     bt[mc][:, 0:C])
                nc.gpsimd.dma_start(out_d[mc * 128:(mc + 1) * 128, 0:C + CD],
                                    ot[:, 0:C + CD])
            for j, mc in enumerate(mcs):
                ot = ots[mc]
                nc.scalar.activation(ot[:, C + CD:], ets[mc][:],
                                     mybir.ActivationFunctionType.Ln,
                                     bias=1.0)
                nc.gpsimd.dma_start(out_d[mc * 128:(mc + 1) * 128, C + CD:],
                                    ot[:, C + CD:])

    nc.compile()
    _cache["nc"] = nc
    return nc


def _prep_shared(W_mu, b_mu, W_sigma, b_sigma, W_pi, b_pi):
    bf16 = ml_dtypes.bfloat16
    # Column order matches the reference output: [logits | loc | scale].
    w_cat = np.concatenate([W_pi, W_mu, W_sigma], axis=-1)      # [G, I, 1040]
    w_np = np.ascontiguousarray(
        w_cat.reshape(NKT, 128, OUT_W).astype(bf16))
    b_cat = np.concatenate([b_pi, b_mu, b_sigma],
                           axis=-1).astype(np.float32)          # [G, 1040]
    return w_np, b_cat


def _core_inputs(x, g, w_np, b_cat, c):
    bf16 = ml_dtypes.bfloat16
    xs = x[c * BLOC:(c + 1) * BLOC]
    gs = g[c * BLOC:(c + 1) * BLOC]
    xT = np.ascontiguousarray(xs.T.astype(bf16))                # [512, 1024]
    gT = gs.T.astype(bf16)                                      # [32, 1024]
    gb = np.ascontiguousarray(
        np.broadcast_to(gT[:, None, :], (G, 128, BLOC)))        # [32,128,1024]
    bias = np.ascontiguousarray(gs.astype(np.float32) @ b_cat)  # [1024, 1040]
    return {"xt": xT, "gb": gb, "w": w_np, "bias": bias}


def kernel(x, g, W_mu, b_mu, W_sigma, b_sigma, W_pi, b_pi):
    nc = _build_program()
    w_np, b_cat = _prep_shared(W_mu, b_mu, W_sigma, b_sigma, W_pi, b_pi)
    in_maps = [_core_inputs(x, g, w_np, b_cat, c) for c in range(NCORES)]
    res = run_bass_kernel_spmd(nc, in_maps, core_ids=list(range(NCORES)))
    out = np.concatenate(
        [res.results[c]["out"] for c in range(NCORES)], axis=0)
    return np.ascontiguousarray(out.astype(np.float32))



# revision 2
# speedup vs baseline: 3.1784x; 3.1784x over previous
"""GroupGMM Trainium2 kernel (fp8 DoubleRow version).

Computes, for B=8192 samples with soft group-mixture weights over G=32 groups:
    logits = einsum("bi,gio,bg->bo", x, W_pi, g) + g @ b_pi        [B, 16]
    loc    = einsum(... W_mu ...)   + g @ b_mu                     [B, 512]
    scale  = softplus(einsum(... W_sigma ...) + g @ b_sigma)+1e-7  [B, 512]
    out    = concat([logits, loc, scale], -1)                      [B, 1040]

Strategy: data-parallel over batch across 8 NeuronCores (1024 rows each).
The group einsum folds into one matmul with contraction K = G*I = 16384 via
z[(g,i),b] = g[b,g]*x[b,i]. Both z and the concatenated weights are
pre-quantized to fp8e4 on the host (scaled by 8 resp. 16 to dodge fp8
subnormals; the 1/128 is folded into the drain ops), so the PE runs
DoubleRow fp8 matmuls: 256-deep contraction per instruction at 0.5
cycles/row — 4x the bf16 matmul rate.

Three column passes over the K dimension, with z cached in SBUF after the
first pass streams it in: sigma (8 PSUM banks, one per 128-sample chunk),
then mu (reusing the banks as sigma drains through ACT's Exp), then pi
(8 chunks packed in one recycled bank). The g@b bias terms are accumulated
directly in PSUM by a small bf16 matmul (K=32) that opens each bank's
accumulation group, so no bias tensors are streamed and no drain-time adds
are needed. softplus = Ln(Exp(v/128) + 1) on ACT; mu/pi drain via DVE
scale-by-1/128 into bf16; all outputs leave in three batched bf16 stores
and the host casts to f32.
"""

import numpy as np
import ml_dtypes

import concourse.bass as bass
import concourse.tile as tile
from concourse import bacc, mybir
from concourse.bass_utils import run_bass_kernel_spmd

B, I, G, C, D = 8192, 512, 32, 16, 32
CD = C * D                      # 512
OUT_W = C + 2 * CD              # 1040
NCORES = 8
BLOC = B // NCORES              # 1024
KTOT = G * I                    # 16384
NDK = KTOT // 256               # 64 double-K tiles (256-deep each)
NMC = BLOC // 128               # 8 sample chunks per core
ZS, WS = 8.0, 16.0              # fp8 pre-scales; drains divide by ZS*WS
SC = ZS * WS

BF16 = mybir.dt.bfloat16
F32 = mybir.dt.float32
FP8 = mybir.dt.float8e4
DR = mybir.MatmulPerfMode.DoubleRow

_cache: dict = {}


def _build_program():
    if "nc" in _cache:
        return _cache["nc"]
    from contextlib import ExitStack

    nc = bacc.Bacc("TRN2", target_bir_lowering=False, debug=False)

    # All host tensors are in "partition-major" layout [128, ...] so every
    # DMA moves multi-KB contiguous runs per partition row.
    z_d = nc.dram_tensor("z", [128, NDK, 2, BLOC], FP8, kind="ExternalInput")
    wmu_d = nc.dram_tensor("wmu", [128, NDK, 2, CD], FP8, kind="ExternalInput")
    wsg_d = nc.dram_tensor("wsg", [128, NDK, 2, CD], FP8, kind="ExternalInput")
    wpi_d = nc.dram_tensor("wpi", [128, NDK, 2, C], FP8, kind="ExternalInput")
    gt_d = nc.dram_tensor("gt", [G, BLOC], BF16, kind="ExternalInput")
    bmu_d = nc.dram_tensor("bmu", [G, CD], BF16, kind="ExternalInput")
    bsg_d = nc.dram_tensor("bsg", [G, CD], BF16, kind="ExternalInput")
    bpi_d = nc.dram_tensor("bpi", [G, C], BF16, kind="ExternalInput")
    out_d = nc.dram_tensor("out", [BLOC, OUT_W], BF16, kind="ExternalOutput")

    with tile.TileContext(nc) as tc, ExitStack() as ctx:
        res = ctx.enter_context(tc.tile_pool(name="res", bufs=1))
        wp = ctx.enter_context(tc.tile_pool(name="wp", bufs=3))
        op = ctx.enter_context(tc.tile_pool(name="op", bufs=1))
        pp = ctx.enter_context(tc.tile_pool(name="pp", bufs=1, space="PSUM"))

        # Small startup loads ahead of the z/w stream on the in-order queue.
        gt = res.tile([G, BLOC], BF16, name="gt", tag="gt")
        nc.sync.dma_start(gt[:], gt_d[:])
        bsg = res.tile([G, CD], BF16, name="bsg", tag="bsg")
        nc.sync.dma_start(bsg[:], bsg_d[:])
        bmu = res.tile([G, CD], BF16, name="bmu", tag="bmu")
        nc.sync.dma_start(bmu[:], bmu_d[:])
        bpi = res.tile([G, C], BF16, name="bpi", tag="bpi")
        nc.sync.dma_start(bpi[:], bpi_d[:])
        wpi = res.tile([128, NDK, 2, C], FP8, name="wpi", tag="wpi")
        nc.sync.dma_start(wpi[:], wpi_d[:])

        # z stays resident across all three passes; streamed in 4-dt slices
        # so the first sigma matmuls start ~3us in.
        zt = res.tile([128, NDK, 2, BLOC], FP8, name="zt", tag="zt")

        QD = 4                   # dk-tiles per DMA
        NQ = NDK // QD

        def chunk(ap, c):
            return ap[:, c * 128:(c + 1) * 128]

        # ---- sigma pass (needs the long Exp/Ln drain, so it goes first:
        # the drain overlaps the mu pass) ----
        psg = {}
        for c in range(NMC):
            psg[c] = pp.tile([128, CD], F32, name=f"psg{c}", tag="acc", bufs=8)
            nc.tensor.matmul(psg[c][:], chunk(gt[:], c), bsg[:],
                             start=True, stop=False)
        for q in range(NQ):
            nc.sync.dma_start(zt[:, q * QD:(q + 1) * QD],
                              z_d[:, q * QD:(q + 1) * QD])
            wt = wp.tile([128, QD, 2, CD], FP8, name=f"wsg{q}", tag="w")
            nc.sync.dma_start(wt[:], wsg_d[:, q * QD:(q + 1) * QD])
            for r in range(QD):
                dt = q * QD + r
                for c in range(NMC):
                    nc.tensor.matmul(
                        psg[c][:], zt[:, dt, :, c * 128:(c + 1) * 128],
                        wt[:, r], start=False, stop=(dt == NDK - 1),
                        perf_mode=DR)

        # sigma drain stage 1: Exp frees each bank for the mu pass.
        ets = {}
        for c in range(NMC):
            et = op.tile([128, CD], F32, name=f"et{c}", tag="et", bufs=8)
            nc.scalar.activation(et[:], psg[c][:],
                                 mybir.ActivationFunctionType.Exp,
                                 scale=1.0 / SC)
            ets[c] = et

        # ---- mu pass ----
        pmu = {}
        for c in range(NMC):
            pmu[c] = pp.tile([128, CD], F32, name=f"pmu{c}", tag="acc", bufs=8)
            nc.tensor.matmul(pmu[c][:], chunk(gt[:], c), bmu[:],
                             start=True, stop=False)
        for q in range(NQ):
            wt = wp.tile([128, QD, 2, CD], FP8, name=f"wmu{q}", tag="w")
            nc.sync.dma_start(wt[:], wmu_d[:, q * QD:(q + 1) * QD])
            for r in range(QD):
                dt = q * QD + r
                for c in range(NMC):
                    nc.tensor.matmul(
                        pmu[c][:], zt[:, dt, :, c * 128:(c + 1) * 128],
                        wt[:, r], start=False, stop=(dt == NDK - 1),
                        perf_mode=DR)

        # sigma drain stage 2 (ACT is free during the mu pass).
        osg = op.tile([128, NMC, CD], BF16, name="osg", tag="osg")
        for c in range(NMC):
            nc.scalar.activation(osg[:, c], ets[c][:],
                                 mybir.ActivationFunctionType.Ln, bias=1.0)
        out_sg = out_d[:, C + CD:].rearrange("(c p) o -> p c o", c=NMC)
        nc.gpsimd.dma_start(out_sg, osg[:])

        # mu drain on DVE (idle otherwise): scale 1/128 + cast to bf16.
        omu = op.tile([128, NMC, CD], BF16, name="omu", tag="omu")
        for c in range(NMC):
            nc.vector.tensor_scalar_mul(omu[:, c], pmu[c][:], 1.0 / SC)
        out_mu = out_d[:, C:C + CD].rearrange("(c p) o -> p c o", c=NMC)
        nc.gpsimd.dma_start(out_mu, omu[:])

        # ---- pi pass: all 8 chunks packed into one recycled PSUM bank ----
        ppi = pp.tile([128, CD], F32, name="ppi", tag="acc", bufs=8)
        for c in range(NMC):
            # start=True on c==0 marks the whole bank pending-zero; later
            # chunks' first writes land on pending-zero bytes.
            nc.tensor.matmul(ppi[:, c * C:(c + 1) * C], chunk(gt[:], c),
                             bpi[:], start=(c == 0), stop=False,
                             skip_group_check=True)
        for dt in range(NDK):
            for c in range(NMC):
                nc.tensor.matmul(
                    ppi[:, c * C:(c + 1) * C],
                    zt[:, dt, :, c * 128:(c + 1) * 128],
                    wpi[:, dt], start=False, stop=(dt == NDK - 1),
                    perf_mode=DR, skip_group_check=True)

        opi = op.tile([128, NMC * C], BF16, name="opi", tag="opi")
        nc.vector.tensor_scalar_mul(opi[:], ppi[:, :NMC * C], 1.0 / SC)
        out_pi = out_d[:, 0:C].rearrange("(c p) o -> p c o", c=NMC)
        nc.gpsimd.dma_start(out_pi, opi[:].rearrange("p (c o) -> p c o", c=NMC))

    nc.compile()
    _cache["nc"] = nc
    return nc


def _prep_shared(W_mu, b_mu, W_sigma, b_sigma, W_pi, b_pi):
    fp8 = ml_dtypes.float8_e4m3
    bf16 = ml_dtypes.bfloat16
    w_cat = np.concatenate([W_pi, W_mu, W_sigma], axis=-1)      # [G, I, 1040]
    # k = g*512 + i -> (dt, j, p); store partition-major [p, dt, j, o].
    w8 = np.ascontiguousarray(
        (w_cat.reshape(NDK, 2, 128, OUT_W) * WS).transpose(2, 0, 1, 3)
    ).astype(fp8)                                               # [128,64,2,1040]
    wpi = np.ascontiguousarray(w8[:, :, :, 0:C])
    wmu = np.ascontiguousarray(w8[:, :, :, C:C + CD])
    wsg = np.ascontiguousarray(w8[:, :, :, C + CD:])
    bmu = (b_mu * SC).astype(bf16)
    bsg = (b_sigma * SC).astype(bf16)
    bpi = (b_pi * SC).astype(bf16)
    return wpi, wmu, wsg, bpi, bmu, bsg


def _core_inputs(x, g, shared, c):
    fp8 = ml_dtypes.float8_e4m3
    bf16 = ml_dtypes.bfloat16
    wpi, wmu, wsg, bpi, bmu, bsg = shared
    xs = x[c * BLOC:(c + 1) * BLOC]
    gs = g[c * BLOC:(c + 1) * BLOC]
    # z[b, k=(g,i)] = g[b,g]*x[b,i], scaled and stored [p, dt, j, b].
    z3 = gs[:, :, None] * xs[:, None, :]                        # [1024, 32, 512]
    z = np.ascontiguousarray(
        (z3.reshape(BLOC, NDK, 2, 128) * ZS).transpose(3, 1, 2, 0)
    ).astype(fp8)                                               # [128,64,2,1024]
    gT = np.ascontiguousarray(gs.T).astype(bf16)                # [32, 1024]
    return {"z": z, "wpi": wpi, "wmu": wmu, "wsg": wsg,
            "gt": gT, "bpi": bpi, "bmu": bmu, "bsg": bsg}


def kernel(x, g, W_mu, b_mu, W_sigma, b_sigma, W_pi, b_pi):
    nc = _build_program()
    shared = _prep_shared(W_mu, b_mu, W_sigma, b_sigma, W_pi, b_pi)
    in_maps = [_core_inputs(x, g, shared, c) for c in range(NCORES)]
    res = run_bass_kernel_spmd(nc, in_maps, core_ids=list(range(NCORES)))
    out = np.concatenate(
        [res.results[c]["out"].astype(np.float32) for c in range(NCORES)],
        axis=0)
    return np.ascontiguousarray(out)
